# revision 3
# baseline (speedup 1.0000x reference)
"""Causal self-attention (B=2, T=2048, C=1024, H=16, D=64) on 8 trn2 cores.

Sharding: data parallel over batch (2) x tensor parallel over heads (4 groups
of 4 heads, Megatron-style). Each core computes qkv projection for its 4
heads, causal attention, and a partial output projection; the host sums the
4 tensor-parallel partials per batch element.

Device-side layouts (per core; everything f32r = fp22-rounded fp32, which runs
matmuls at full bf16 rate when the moving free dim >= 256):
  xT    [1024c, 2048t]   x[b] transposed (host)
  wqkvT [1024c, 768]     cols 0:256 q, 256:512 k, 512:768 v (head-slice, host)
  woT   [256c, 1024o]    w_out column-slice transposed (host)
  qT,kT [128, 2, 2048]   d-tile (head pair) x t;  pair p rows: head 2p -> 0:64,
                         head 2p+1 -> 64:128
  v     [128, 16, 260]   t-tile x [v(64) | ones(1)] x 4 heads (ones col fuels
                         the sumexp row of the AV matmul)
  scores^T in psum as [tk=128, tq<=1024] blocks; softmax without max
  subtraction (|s| <= |q||k|/8 is tiny); exp on ACT with fused 1/8 scale;
  causal handled by skipping fully-masked tk blocks + 4 diagonal masks.
"""

import numpy as np

B, T, C = 2, 2048, 1024
H, D = 16, 64
NCORES = 8
TP = 4          # head groups (tensor parallel)
DL = (H // TP) * D  # 256 local channels per core

_CACHE: dict = {}


def _make_consts() -> np.ndarray:
    consts = np.zeros((128, 2176), dtype=np.float32)
    p = np.arange(128)[:, None]
    f = np.arange(128)[None, :]
    # aligned triangular band mask: every diagonal block's maskable region is
    # the 128x128 block at tq-offset 128m, where the mask is (f >= p)
    consts[:, 0:128] = (f >= p).astype(np.float32)
    consts[:, 2048:2112] = 1.0   # ones block
    consts[64, 2112:2176] = 1.0  # selector (row 64)
    return consts


def _build(loop_n: int = 1, bench_io: bool = False, phases: str = "all",
           loop_r: int = 1):
    import contextlib
    import concourse.mybir as mybir
    import concourse.tile as tile
    from concourse import bacc

    F32 = mybir.dt.float32
    F32R = mybir.dt.float32r
    BF16 = mybir.dt.bfloat16
    EXP = mybir.ActivationFunctionType.Exp

    nc = bacc.Bacc("TRN2", target_bir_lowering=False, debug=False,
                   num_devices=NCORES)
    if bench_io:
        # timing-only build: big operands are internal (uninitialized) DRAM so
        # per-call host->device shipping is tiny; consts stay real
        xT = nc.dram_tensor("xT_i", [C, T], F32R)
        wqkvT = nc.dram_tensor("wqkvT_i", [C, 3 * DL], F32R)
        woT = nc.dram_tensor("woT_i", [DL, C], F32R)
    else:
        xT = nc.declare_dram_parameter("xT", [C, T], F32R, isOutput=False)
        wqkvT = nc.declare_dram_parameter("wqkvT", [C, 3 * DL], F32R,
                                          isOutput=False)
        woT = nc.declare_dram_parameter("woT", [DL, C], F32R, isOutput=False)
    consts = nc.declare_dram_parameter("consts", [128, 2176], F32R, isOutput=False)
    y = nc.declare_dram_parameter("y", [T, C], F32, isOutput=True)

    xT_r = xT[:].rearrange("(o p) t -> p o t", p=128)        # [128, 8, 2048]
    wqkvT_r = wqkvT[:].rearrange("(o p) f -> p o f", p=128)  # [128, 8, 768]
    woT_r = woT[:].rearrange("(o p) f -> p o f", p=128)      # [128, 2, 1024]
    y_r = y[:].rearrange("(m p) f -> p m f", p=128)          # [128, 16, 1024]

    with tile.TileContext(nc) as tc:
        with tc.tile_pool(name="persist", bufs=1) as sbP, \
             tc.tile_pool(name="work", bufs=1) as sbW, \
             tc.tile_pool(name="ps", bufs=1, space="PSUM") as ps:

            xT_sb = sbP.tile([128, 8, T], F32R)
            wqkv_sb = sbP.tile([128, 8, 3 * DL], F32R)
            wo_sb = sbP.tile([128, 2, C], F32R)
            c_sb = sbP.tile([128, 2176], F32R)
            qT_sb = sbP.tile([128, 2, T], F32R)
            kT_sb = sbP.tile([128, 2, T], F32R)
            v_sb = sbP.tile([128, 16, 260], BF16)
            oT_sb = sbP.tile([128, 2, T], F32R)

            # split input loads across the two HWDGE queues (SP + ACT);
            # xT tq-major so the first qk/v groups can start after 1/4 of x
            for o in range(8):
                weng = nc.scalar if o % 2 == 0 else nc.sync
                weng.dma_start(out=wqkv_sb[:, o], in_=wqkvT_r[:, o])
            for tq in range(4):
                for o in range(8):
                    qeng = nc.sync if o % 2 == 0 else nc.scalar
                    qeng.dma_start(
                        out=xT_sb[:, o, 512 * tq:512 * tq + 512],
                        in_=xT_r[:, o, 512 * tq:512 * tq + 512])
            nc.scalar.dma_start(out=wo_sb[:], in_=woT_r[:])
            nc.sync.dma_start(out=c_sb[:], in_=consts[:])
            # ones columns of v: col 64 of each 65-wide head group
            # (DVE copy converts f32r consts -> bf16)
            v_ones = v_sb[:].rearrange("p t (h c) -> p t h c", h=4)[:, :, :, 64:65]
            ones_src = c_sb[:, 2048:2112].rearrange("p (t h) -> p t h", t=16)
            nc.vector.tensor_copy(out=v_ones, in_=ones_src.unsqueeze(3))


            def qk_group(pair, wq_off, dst_sb, tq):
                pt = ps.tile([128, 512], F32, tag="yps", bufs=2, name="qk_ps")
                col = wq_off + 128 * pair
                for kc in range(8):
                    nc.tensor.matmul(pt[:],
                                     wqkv_sb[:, kc, col:col + 128],
                                     xT_sb[:, kc, 512 * tq:512 * tq + 512],
                                     start=(kc == 0), stop=(kc == 7))
                nc.vector.tensor_copy(
                    out=dst_sb[:, pair, 512 * tq:512 * tq + 512], in_=pt[:])

            def v_group(ti):
                pt = ps.tile([128, 512], F32, tag="yps", bufs=2, name="v_ps")
                for kc in range(8):
                    nc.tensor.matmul(pt[:, 0:256],
                                     xT_sb[:, kc, 128 * ti:128 * ti + 128],
                                     wqkv_sb[:, kc, 512:768],
                                     start=(kc == 0), stop=(kc == 7))
                v_dst = v_sb[:, ti].rearrange("p (h c) -> p h c", h=4)[:, :, 0:64]
                nc.vector.tensor_copy(
                    out=v_dst,
                    in_=pt[:, 0:256].rearrange("p (h c) -> p h c", h=4))

            def oproj_slice(j):
                for m in range(4 * j, 4 * j + 4):
                    for nb in (0, 1):
                        yp = ps.tile([128, 512], F32, tag="yps", bufs=2,
                                     name="y_ps")
                        for ct in (0, 1):
                            nc.tensor.matmul(
                                yp[:],
                                oT_sb[:, ct, 128 * m:128 * m + 128],
                                wo_sb[:, ct, 512 * nb:512 * nb + 512],
                                start=(ct == 0), stop=(ct == 1))
                        yst = sbW.tile([128, 512], F32, tag="yst", bufs=3,
                                       name="y_st")
                        nc.vector.tensor_copy(out=yst[:], in_=yp[:])
                        nc.sync.dma_start(
                            out=y_r[:, m, 512 * nb:512 * nb + 512], in_=yst[:])

            def attn_pair(pair, interleave, pre_j=None):
                # pair 1 runs j descending so the small j=0 (2 chunks) is the
                # exposed tail instead of j=3 (8 chunks of ACT exp work)
                j_order = range(4) if pair == 0 else (3, 2, 1, 0)
                for j in j_order:
                    if pre_j is not None:
                        pre_j(j)
                    acc = {}
                    for h in (0, 1):
                        acc[h] = ps.tile([128, 512], F32, tag="acc", bufs=2,
                                         name="av_acc")
                    nch = 2 * j + 2
                    for cnk in range(nch):
                        i0 = 2 * cnk
                        s_ps = {}
                        for h in (0, 1):
                            base = 64 * h
                            sp = ps.tile([128, 1024], F32, tag="spsum", bufs=2,
                                         name="s_ps")
                            s_ps[h] = sp
                            for s in (0, 1):
                                i = i0 + s
                                nc.tensor.matmul(
                                    sp[:, 512 * s:512 * s + 512],
                                    kT_sb[base:base + 64, pair,
                                          128 * i:128 * i + 128],
                                    qT_sb[base:base + 64, pair,
                                          512 * j:512 * j + 512],
                                    start=True, stop=True)
                        diag = cnk >= 2 * j
                        for h in (0, 1):
                            e = sbW.tile([128, 1024], BF16, tag="expS", bufs=8,
                                         name="expS")
                            if not diag or i0 - 4 * j == 0:
                                # non-diag, or the m=(0,1) diagonal chunk: one
                                # full-width exp (the m=1 masked prefix is
                                # finite and never read by the shrunk AV)
                                nc.scalar.activation(out=e[:], in_=s_ps[h][:],
                                                     func=EXP, scale=0.125)
                                if diag:
                                    for s in (0, 1):
                                        m = i0 + s - 4 * j
                                        off = 512 * s + 128 * m
                                        nc.vector.tensor_mul(
                                            out=e[:, off:off + 128],
                                            in0=e[:, off:off + 128],
                                            in1=c_sb[:, 0:128])
                            else:
                                # per half: cols [0, 128m) are fully masked
                                # (never read by the shrunk AV matmul); the
                                # diagonal band [128m, 128m+128) gets the
                                # aligned triangular mask.
                                for s in (0, 1):
                                    m = i0 + s - 4 * j
                                    off = 512 * s + 128 * m
                                    w = 512 - 128 * m
                                    nc.scalar.activation(
                                        out=e[:, off:off + w],
                                        in_=s_ps[h][:, off:off + w],
                                        func=EXP, scale=0.125)
                                    nc.vector.tensor_mul(
                                        out=e[:, off:off + 128],
                                        in0=e[:, off:off + 128],
                                        in1=c_sb[:, 0:128])
                            hc = 2 * pair + h
                            vh = v_sb[:].rearrange("p t (h c) -> p t h c", h=4)
                            for s in (0, 1):
                                i = i0 + s
                                tq0 = 128 * (i - 4 * j) if diag else 0
                                nc.tensor.matmul(
                                    acc[h][0:65, tq0:512],
                                    vh[:, i, hc, :],
                                    e[:, 512 * s + tq0:512 * s + 512],
                                    start=(cnk == 0 and s == 0),
                                    stop=(cnk == nch - 1 and s == 1))
                        interleave()
                    # normalize + evict into oT_sb
                    for h in (0, 1):
                        st = sbW.tile([65, 512], F32R, tag="st", bufs=4,
                                      name="st")
                        nc.vector.tensor_copy(out=st[:], in_=acc[h][0:65, :])
                        with nc.allow_low_precision(reason="softmax recip"):
                            nc.vector.reciprocal(out=st[64:65, :],
                                                 in_=st[64:65, :])
                        bc = ps.tile([128, 512], F32, tag="yps", bufs=2,
                                     name="bc_ps")
                        nc.tensor.matmul(bc[0:64, 0:512],
                                         c_sb[0:65, 2112:2176],
                                         st[0:65, :], start=True, stop=True)
                        dst = oT_sb[64 * h:64 * h + 64, pair,
                                    512 * j:512 * j + 512]
                        if h == 0:
                            nc.vector.tensor_mul(out=dst, in0=st[0:64, :],
                                                 in1=bc[0:64, 0:512])
                        else:
                            nc.vector.tensor_mul(out=st[0:64, :],
                                                 in0=st[0:64, :],
                                                 in1=bc[0:64, 0:512])
                            nc.sync.dma_start(out=dst, in_=st[0:64, :])
                    if pair == 1 and phases != "noproj":
                        oproj_slice(j)

            def emit_all():
                # phase A: qk pair 0 + the first v tiles attention j=0 needs
                for wq_off, dst in ((0, qT_sb), (DL, kT_sb)):
                    for tq in range(4):
                        qk_group(0, wq_off, dst, tq)
                for ti in range(4):
                    v_group(ti)

                # phase B: attention pair 0, with the remaining v tiles and
                # the pair-1 qk projection interleaved (v first — attention
                # j consumes v tiles up to 4j+3)
                fill = [("v", ti) for ti in range(4, 16)]
                fill += [("qk", wq_off, dst, tq)
                         for wq_off, dst in ((0, qT_sb), (DL, kT_sb))
                         for tq in range(4)]
                state = {"i": 0, "vmax": 3}

                def emit_fill_item():
                    if state["i"] < len(fill):
                        item = fill[state["i"]]
                        if item[0] == "v":
                            v_group(item[1])
                            state["vmax"] = item[1]
                        else:
                            qk_group(1, item[1], item[2], item[3])
                        state["i"] += 1
                        return True
                    return False

                def need_v(j):
                    # attention j reads v tiles up to 4j+3: ensure they are
                    # emitted (program order = dependency order in Tile)
                    while state["vmax"] < 4 * j + 3:
                        emit_fill_item()

                if phases == "qkv":
                    while emit_fill_item():
                        pass
                    return

                attn_pair(0, emit_fill_item, pre_j=need_v)
                while emit_fill_item():
                    pass

                # phase C: attention pair 1 (out-proj slices emitted per j)
                attn_pair(1, lambda: None)

            if loop_r > 1:
                # device-side repeat for benchmarking: body = loop_n unrolled
                # iterations, repeated loop_r times via a Tile For loop
                with tc.For_i(0, loop_r, 1):
                    for _rep in range(loop_n):
                        emit_all()
            else:
                for _rep in range(loop_n):
                    emit_all()

    nc.compile()
    return nc


def _get_nc():
    if "nc" not in _CACHE:
        _CACHE["nc"] = _build()
    return _CACHE["nc"]


def _in_maps(x, w_qkv, w_out):
    consts = _make_consts()
    maps = []
    for core in range(NCORES):
        b, r = core // TP, core % TP
        xTc = np.ascontiguousarray(x[b].T)
        wq = w_qkv[DL * r:DL * r + DL]
        wk = w_qkv[C + DL * r:C + DL * r + DL]
        wv = w_qkv[2 * C + DL * r:2 * C + DL * r + DL]
        wqkvT = np.ascontiguousarray(np.concatenate([wq, wk, wv], axis=0).T)
        woT = np.ascontiguousarray(w_out[:, DL * r:DL * r + DL].T)
        maps.append({"xT": xTc, "wqkvT": wqkvT, "woT": woT, "consts": consts})
    return maps


def _run(x, w_qkv, w_out, trace=False):
    from concourse.bass_utils import run_bass_kernel_spmd
    nc = _get_nc()
    res = run_bass_kernel_spmd(nc, _in_maps(x, w_qkv, w_out),
                               list(range(NCORES)), trace=trace)
    y = np.zeros((B, T, C), dtype=np.float32)
    for core in range(NCORES):
        y[core // TP] += res.results[core]["y"]
    return y, res


def kernel(x, w_qkv, w_out):
    x = np.asarray(x, dtype=np.float32)
    w_qkv = np.asarray(w_qkv, dtype=np.float32)
    w_out = np.asarray(w_out, dtype=np.float32)
    y, _ = _run(x, w_qkv, w_out)
    return y



# revision 10
# speedup vs baseline: 5.3359x; 5.3359x over previous
"""Causal self-attention (B=2, T=2048, C=1024, H=16, D=64) on 8 trn2 cores.

Sharding: data parallel over batch (2) x tensor parallel over heads (4 groups
of 4 heads, Megatron-style). Each core computes qkv projection for its 4
heads, causal attention, and a partial output projection; the host sums the
4 tensor-parallel partials per batch element.

Device-side layouts (per core):
  xT    [1024c, 2048t]   x[b] transposed (host), f32r
  wqkvT [1024c, 768]     cols 0:256 q, 256:512 k, 512:768 v (head-slice, host)
  woT   [256c, 1024o]    w_out column-slice transposed (host)
  qT,kT [128, 2, 2048]   bf16 d-tile (head pair) x t; pair p rows:
                         head 2p -> 0:64, head 2p+1 -> 64:128
  v     [128, 16, 260]   t-tile x [v(64) | ones(1)] x 4 heads (ones col fuels
                         the sumexp row of the AV matmul), bf16
  scores^T in psum as one [tk=128, 1024] tile per tk-block: cols 0:512 head
  h0, 512:1024 head h1 over a 512-wide tq slice; softmax without max
  subtraction (|s| <= |q||k|/8 is small); exp on ACT with fused 1/8 scale;
  causal handled by skipping fully-masked tk blocks, trimming the scores/
  exp/AV column range on diagonal blocks, and a 128x128 triangular band
  mask applied on the (otherwise idle) GpSimd/Pool engine so DVE/ACT stay
  off the critical path. y is evicted from PSUM in bf16 (halves DVE copy
  cost + DMA bytes); the host accumulates partials in f32.

Engine budget per iteration (2.4 GHz warm): PE ~108us, ACT ~67us, DVE ~50us,
Pool ~25us. The scheduling goal is PE >= 95% busy so the HAM clock gate
stays open (cold PE runs at 1.2 GHz and doubles the runtime).
"""

import numpy as np

B, T, C = 2, 2048, 1024
H, D = 16, 64
NCORES = 8
TP = 4          # head groups (tensor parallel)
DL = (H // TP) * D  # 256 local channels per core

_CACHE: dict = {}


def _make_consts() -> np.ndarray:
    consts = np.zeros((128, 2176), dtype=np.float32)
    p = np.arange(128)[:, None]
    f = np.arange(128)[None, :]
    # aligned triangular band mask: every diagonal block's maskable region is
    # the 128x128 block at tq-offset 128m, where the mask is (f >= p)
    consts[:, 0:128] = (f >= p).astype(np.float32)
    consts[:, 2048:2112] = 1.0   # ones block
    consts[64, 2112:2176] = 1.0  # selector (row 64)
    return consts


def _build(loop_n: int = 1, bench_io: bool = False, phases: str = "all",
           loop_r: int = 1):
    import concourse.mybir as mybir
    import concourse.tile as tile
    from concourse import bacc

    F32 = mybir.dt.float32
    F32R = mybir.dt.float32r
    BF16 = mybir.dt.bfloat16
    EXP = mybir.ActivationFunctionType.Exp

    nc = bacc.Bacc("TRN2", target_bir_lowering=False, debug=False,
                   num_devices=NCORES)
    if bench_io:
        # timing-only build: big operands are internal (uninitialized) DRAM so
        # per-call host->device shipping is tiny; consts stay real
        xT = nc.dram_tensor("xT_i", [C, T], F32R)
        wqkvT = nc.dram_tensor("wqkvT_i", [C, 3 * DL], F32R)
        woT = nc.dram_tensor("woT_i", [DL, C], F32R)
    else:
        xT = nc.declare_dram_parameter("xT", [C, T], F32R, isOutput=False)
        wqkvT = nc.declare_dram_parameter("wqkvT", [C, 3 * DL], F32R,
                                          isOutput=False)
        woT = nc.declare_dram_parameter("woT", [DL, C], F32R, isOutput=False)
    consts = nc.declare_dram_parameter("consts", [128, 2176], F32R, isOutput=False)
    y = nc.declare_dram_parameter("y", [T, C], BF16, isOutput=True)

    xT_r = xT[:].rearrange("(o p) t -> p o t", p=128)        # [128, 8, 2048]
    wqkvT_r = wqkvT[:].rearrange("(o p) f -> p o f", p=128)  # [128, 8, 768]
    woT_r = woT[:].rearrange("(o p) f -> p o f", p=128)      # [128, 2, 1024]
    y_r = y[:].rearrange("(m p) f -> p m f", p=128)          # [128, 16, 1024]

    with tile.TileContext(nc) as tc:
        with tc.tile_pool(name="persist", bufs=1) as sbP, \
             tc.tile_pool(name="work", bufs=1) as sbW, \
             tc.tile_pool(name="ps", bufs=1, space="PSUM") as ps:

            xT_sb = sbP.tile([128, 8, T], F32R)
            wqkv_sb = sbP.tile([128, 8, 3 * DL], F32R)
            wo_sb = sbP.tile([128, 2, C], F32R)
            c_sb = sbP.tile([128, 2176], F32R)
            band_sb = sbP.tile([128, 128], BF16)
            qT_sb = sbP.tile([128, 2, T], BF16)
            kT_sb = sbP.tile([128, 2, T], BF16)
            v_sb = sbP.tile([128, 16, 260], BF16)
            oT_sb = sbP.tile([128, 2, T], BF16)

            # split input loads across the two HWDGE queues (SP + ACT);
            # xT tq-major so the first qk/v groups can start after 1/4 of x
            for o in range(8):
                weng = nc.scalar if o % 2 == 0 else nc.sync
                weng.dma_start(out=wqkv_sb[:, o], in_=wqkvT_r[:, o])
            for tq in range(4):
                for o in range(8):
                    qeng = nc.sync if o % 2 == 0 else nc.scalar
                    qeng.dma_start(
                        out=xT_sb[:, o, 512 * tq:512 * tq + 512],
                        in_=xT_r[:, o, 512 * tq:512 * tq + 512])
            nc.scalar.dma_start(out=wo_sb[:], in_=woT_r[:])
            nc.sync.dma_start(out=c_sb[:], in_=consts[:])
            # ones columns of v: col 64 of each 65-wide head group
            # (DVE copy converts f32r consts -> bf16)
            v_ones = v_sb[:].rearrange("p t (h c) -> p t h c", h=4)[:, :, :, 64:65]
            ones_src = c_sb[:, 2048:2112].rearrange("p (t h) -> p t h", t=16)
            nc.vector.tensor_copy(out=v_ones, in_=ones_src.unsqueeze(3))
            # bf16 copy of the triangular band mask for the Pool engine
            nc.vector.tensor_copy(out=band_sb[:], in_=c_sb[:, 0:128])


            def qk_group(pair, wq_off, dst_sb, tq):
                pt = ps.tile([128, 512], F32, tag="yps", bufs=2, name="qk_ps")
                col = wq_off + 128 * pair
                for kc in range(8):
                    nc.tensor.matmul(pt[:],
                                     wqkv_sb[:, kc, col:col + 128],
                                     xT_sb[:, kc, 512 * tq:512 * tq + 512],
                                     start=(kc == 0), stop=(kc == 7))
                nc.vector.tensor_copy(
                    out=dst_sb[:, pair, 512 * tq:512 * tq + 512], in_=pt[:])

            def v_group(ti):
                pt = ps.tile([128, 512], F32, tag="yps", bufs=2, name="v_ps")
                for kc in range(8):
                    nc.tensor.matmul(pt[:, 0:256],
                                     xT_sb[:, kc, 128 * ti:128 * ti + 128],
                                     wqkv_sb[:, kc, 512:768],
                                     start=(kc == 0), stop=(kc == 7))
                v_dst = v_sb[:, ti].rearrange("p (h c) -> p h c", h=4)[:, :, 0:64]
                nc.vector.tensor_copy(
                    out=v_dst,
                    in_=pt[:, 0:256].rearrange("p (h c) -> p h c", h=4))

            def y_tile(m, nb):
                yp = ps.tile([128, 512], F32, tag="yps", bufs=2, name="y_ps")
                for ct in (0, 1):
                    nc.tensor.matmul(
                        yp[:],
                        oT_sb[:, ct, 128 * m:128 * m + 128],
                        wo_sb[:, ct, 512 * nb:512 * nb + 512],
                        start=(ct == 0), stop=(ct == 1))
                yst = sbW.tile([128, 512], BF16, tag="yst", bufs=6,
                               name="y_st")
                nc.vector.tensor_copy(out=yst[:], in_=yp[:])
                nc.sync.dma_start(
                    out=y_r[:, m, 512 * nb:512 * nb + 512], in_=yst[:])

            vh = v_sb[:].rearrange("p t (h c) -> p t h c", h=4)

            def attn_pair(pair, interleave, pre_j=None, post_j=None):
                for j in range(4):
                    if pre_j is not None:
                        pre_j(j)
                    acc = {}
                    for h in (0, 1):
                        acc[h] = ps.tile([128, 512], F32, tag="acc", bufs=2,
                                         name="av_acc")
                    nch = 4 * j + 4
                    for i in range(nch):
                        m = i - 4 * j   # >= 0 on diagonal tk blocks
                        off = 128 * m if m >= 1 else 0
                        sp = ps.tile([128, 1024], F32, tag="spsum", bufs=2,
                                     name="s_ps")
                        for h in (0, 1):
                            base = 64 * h
                            nc.tensor.matmul(
                                sp[:, 512 * h + off:512 * h + 512],
                                kT_sb[base:base + 64, pair,
                                      128 * i:128 * i + 128],
                                qT_sb[base:base + 64, pair,
                                      512 * j + off:512 * j + 512],
                                start=True, stop=True)
                        e = sbW.tile([128, 1024], BF16, tag="expS", bufs=8,
                                     name="expS")
                        if m < 1:
                            # one full-width exp covers both heads
                            nc.scalar.activation(out=e[:], in_=sp[:],
                                                 func=EXP, scale=0.125)
                        else:
                            for h in (0, 1):
                                o = 512 * h + off
                                w = 512 - off
                                nc.scalar.activation(
                                    out=e[:, o:o + w], in_=sp[:, o:o + w],
                                    func=EXP, scale=0.125)
                        if m >= 0:
                            # triangular band mask on the idle Pool engine
                            for h in (0, 1):
                                o = 512 * h + off
                                nc.gpsimd.tensor_mul(
                                    out=e[:, o:o + 128],
                                    in0=e[:, o:o + 128],
                                    in1=band_sb[:])
                        tq0 = off
                        last = i == nch - 1
                        for h in (0, 1):
                            lhs = vh[:, i, 2 * pair + h, :]
                            if m >= 0 and m < 3:
                                # split AV: the part beyond the masked band
                                # depends only on exp; the 128-wide band part
                                # additionally waits the Pool mask
                                nc.tensor.matmul(
                                    acc[h][0:65, tq0 + 128:512], lhs,
                                    e[:, 512 * h + tq0 + 128:512 * h + 512],
                                    start=(i == 0), stop=False)
                                nc.tensor.matmul(
                                    acc[h][0:65, tq0:tq0 + 128], lhs,
                                    e[:, 512 * h + tq0:512 * h + tq0 + 128],
                                    start=(i == 0), stop=last)
                            else:
                                nc.tensor.matmul(
                                    acc[h][0:65, tq0:512], lhs,
                                    e[:, 512 * h + tq0:512 * h + 512],
                                    start=(i == 0), stop=last)
                        interleave(i)
                    # normalize + evict into oT_sb (bf16 path: halves the
                    # j-boundary DVE burst; st copies high-priority so the
                    # acc psum banks recycle before the next j's AV needs
                    # them)
                    for h in (0, 1):
                        st = sbW.tile([65, 512], BF16, tag="st", bufs=4,
                                      name="st")
                        with tc.high_priority():
                            nc.vector.tensor_copy(out=st[:],
                                                  in_=acc[h][0:65, :])
                        with nc.allow_low_precision(reason="softmax recip"):
                            nc.vector.reciprocal(out=st[64:65, :],
                                                 in_=st[64:65, :])
                        bc = ps.tile([128, 512], F32, tag="yps", bufs=2,
                                     name="bc_ps")
                        nc.tensor.matmul(bc[0:64, 0:512],
                                         c_sb[0:65, 2112:2176],
                                         st[0:65, :], start=True, stop=True)
                        dst = oT_sb[64 * h:64 * h + 64, pair,
                                    512 * j:512 * j + 512]
                        if h == 0:
                            nc.vector.tensor_mul(out=dst, in0=st[0:64, :],
                                                 in1=bc[0:64, 0:512])
                        else:
                            nc.vector.tensor_mul(out=st[0:64, :],
                                                 in0=st[0:64, :],
                                                 in1=bc[0:64, 0:512])
                            nc.sync.dma_start(out=dst, in_=st[0:64, :])
                    if post_j is not None:
                        post_j(j)

            def emit_all():
                # phase A: qk pair 0 + the first v tiles attention j=0 needs
                for wq_off, dst in ((0, qT_sb), (DL, kT_sb)):
                    for tq in range(4):
                        qk_group(0, wq_off, dst, tq)
                for ti in range(4):
                    v_group(ti)

                # phase B: attention pair 0, with the remaining v tiles and
                # the pair-1 qk projection for tq=0 interleaved, paced one
                # item per other chunk so the DVE/psum load stays smooth
                fill = [("v", ti) for ti in range(4, 16)]
                fill += [("qk", wq_off, dst, 0)
                         for wq_off, dst in ((0, qT_sb), (DL, kT_sb))]
                state = {"i": 0, "vmax": 3}

                def emit_fill_item():
                    if state["i"] < len(fill):
                        item = fill[state["i"]]
                        if item[0] == "v":
                            v_group(item[1])
                            state["vmax"] = item[1]
                        else:
                            qk_group(1, item[1], item[2], item[3])
                        state["i"] += 1
                        return True
                    return False

                def need_v(j):
                    # attention j reads v tiles up to 4j+3: ensure they are
                    # emitted (program order = dependency order in Tile)
                    while state["vmax"] < 4 * j + 3:
                        emit_fill_item()

                if phases == "qkv":
                    while emit_fill_item():
                        pass
                    for wq_off, dst in ((0, qT_sb), (DL, kT_sb)):
                        for tq in range(1, 4):
                            qk_group(1, wq_off, dst, tq)
                    return

                attn_pair(0, lambda i: emit_fill_item() if i % 2 else None,
                          pre_j=need_v)
                while emit_fill_item():
                    pass

                # phase C: attention pair 1 (ascending j). Fill inventory:
                # the pair-1 qk groups for tq=j+1 (prereq of the next j) and
                # the out-projection tiles of completed j's, paced to avoid
                # both bunching at j starts and running dry at j ends.
                qk_pend = [(tq, wq_off, dst)
                           for tq in range(1, 4)
                           for wq_off, dst in ((0, qT_sb), (DL, kT_sb))]
                y_pend = []

                def pop_fill(i):
                    if qk_pend:
                        tq, wq_off, dst = qk_pend.pop(0)
                        qk_group(1, wq_off, dst, tq)
                    elif y_pend and phases != "noproj" and (
                            i % 2 or len(y_pend) > 4):
                        y_pend.pop(0)()

                def need_qk(j):
                    while qk_pend and qk_pend[0][0] <= j:
                        tq, wq_off, dst = qk_pend.pop(0)
                        qk_group(1, wq_off, dst, tq)

                def push_oproj(j):
                    for mm in range(4 * j, 4 * j + 4):
                        for nb in (0, 1):
                            y_pend.append(lambda m=mm, n=nb: y_tile(m, n))

                attn_pair(1, pop_fill, pre_j=need_qk, post_j=push_oproj)
                if phases != "noproj":
                    while y_pend:
                        y_pend.pop(0)()

            if loop_r > 1:
                # device-side repeat for benchmarking: body = loop_n unrolled
                # iterations, repeated loop_r times via a Tile For loop
                with tc.For_i(0, loop_r, 1):
                    for _rep in range(loop_n):
                        emit_all()
            else:
                for _rep in range(loop_n):
                    emit_all()

    nc.compile()
    return nc


def _get_nc():
    if "nc" not in _CACHE:
        _CACHE["nc"] = _build()
    return _CACHE["nc"]


def _in_maps(x, w_qkv, w_out):
    consts = _make_consts()
    maps = []
    for core in range(NCORES):
        b, r = core // TP, core % TP
        xTc = np.ascontiguousarray(x[b].T)
        wq = w_qkv[DL * r:DL * r + DL]
        wk = w_qkv[C + DL * r:C + DL * r + DL]
        wv = w_qkv[2 * C + DL * r:2 * C + DL * r + DL]
        wqkvT = np.ascontiguousarray(np.concatenate([wq, wk, wv], axis=0).T)
        woT = np.ascontiguousarray(w_out[:, DL * r:DL * r + DL].T)
        maps.append({"xT": xTc, "wqkvT": wqkvT, "woT": woT, "consts": consts})
    return maps


def _run(x, w_qkv, w_out, trace=False):
    from concourse.bass_utils import run_bass_kernel_spmd
    nc = _get_nc()
    res = run_bass_kernel_spmd(nc, _in_maps(x, w_qkv, w_out),
                               list(range(NCORES)), trace=trace)
    y = np.zeros((B, T, C), dtype=np.float32)
    for core in range(NCORES):
        y[core // TP] += np.asarray(res.results[core]["y"], dtype=np.float32)
    return y, res


def kernel(x, w_qkv, w_out):
    x = np.asarray(x, dtype=np.float32)
    w_qkv = np.asarray(w_qkv, dtype=np.float32)
    w_out = np.asarray(w_out, dtype=np.float32)
    y, _ = _run(x, w_qkv, w_out)
    return y
